# revision 17
# baseline (speedup 1.0000x reference)
"""Trainium2 Bass kernel for nn_EntropyConv (masked 5x5 PixelCNN-style conv,
per-latitude-partition padding + width masking + PReLU).

Strategy: data-parallel over batch (8 cores x 1 batch element). Per core,
a row-phase-split SBUF layout puts (row mod 4, ci) on the 128 K-partitions
so each PSUM tile computes 4 output rows x 32 channels. The PixelCNN mask
zeroes kh=3,4 entirely, so each output row only needs input rows r-2..r.
Window-1 (rows 4b-2..4b+1) takes 5 kw-shift matmuls; the window-2 taps
(rows 4b+2,4b+3 -> 11 weight blocks) are packed into just 2 matmuls using
duplicate tiles that bake the kw column shift into the storage offset,
for 7 matmuls per tile instead of 10 (the packing floor: 28 distinct
(row, kw) slot-pairs / 4 slots per matmul).

All tile images (x4 window-1, x2a/x2b window-2 with baked shifts, guard
zeros included) are assembled on the host in bf16, so the device issues
only 3 input DMAs per latitude chunk. Matmuls are bf16 (PSUM fp32);
output is stored bf16 and upcast on host (rel tolerance 2e-2).
"""

import sys
from contextlib import ExitStack

import numpy as np
import ml_dtypes

sys.path.insert(0, "/opt/trn_rl_repo")

import concourse.bass as bass  # noqa: E402
import concourse.tile as tile  # noqa: E402
from concourse import bacc, mybir  # noqa: E402
from concourse.bass_utils import run_bass_kernel_spmd  # noqa: E402

# Model constants (hardcoded per problem spec)
NGROUPS, CIN, COUT, KSIZE, NPART = 8, 4, 4, 5, 8
B, H, W = 8, 256, 512
CI = NGROUPS * CIN   # 32
CO = NGROUPS * COUT  # 32
Hp = H // NPART      # 32 rows per latitude chunk
NBLK = Hp // 4       # 8 four-row blocks per chunk
NCORES = 8
F32 = mybir.dt.float32
BF16 = mybir.dt.bfloat16
NPBF16 = ml_dtypes.bfloat16

XLEN = NBLK * W + 8  # SBUF x tiles: 8 blocks at pitch W, +8 slack

NW = 7  # matmuls per 4-row tile: 5 window-1 kw shifts + 2 packed window-2

LAST_RESULT = None  # BassKernelResults from the most recent run (for test.py)


def _group_mask():
    """PixelCNN group mask for 5x5 kernel, mask-B (hidden) variant."""
    m = np.zeros((CO, CI, KSIZE, KSIZE), np.float32)
    c = KSIZE // 2
    m[:, :, :c, :] = 1.0
    m[:, :, c, :c] = 1.0
    gin = np.arange(CI) // CIN
    gout = np.arange(CO) // COUT
    center = gin[None, :] <= gout[:, None]
    m[:, :, c, c] = center.astype(np.float32)
    return m


def _build_weights(weight):
    """lhsT stack [7, 128, 128]: 5 window-1 block-Toeplitz matrices (one
    per kw) + 2 packed window-2 matrices.

    w[kw, 32*rp+ci, 4*co+j]: input row (4hb+rp-2) -> output row (4hb+j),
    kh = rp-j (mask zeroes kh >= 3).
    wA slot k (k=0..3) = row 4hb+2 @ kw=k:  j=2 <- wm[2,k], j=3 <- wm[1,k]
    wB slot 0          = row 4hb+2 @ kw=4:  j=3 <- wm[1,4]
    wB slot s (s=1..3) = row 4hb+3 @ kw=s-1: j=3 <- wm[2,s-1]
    """
    wm = (weight * _group_mask()).astype(np.float32)  # [co, ci, kh, kw]
    wt = np.zeros((NW, 128, 128), np.float32)
    for kw in range(KSIZE):
        for rp in range(4):
            for j in range(4):
                kh = rp - j
                if 0 <= kh < KSIZE:
                    wt[kw, 32 * rp:32 * rp + 32, j::4] = wm[:, :, kh, kw].T
    for k in range(4):  # wA slots: row+2 @ kw=k
        wt[5, 32 * k:32 * k + 32, 2::4] = wm[:, :, 2, k].T  # j=2, kh=2
        wt[5, 32 * k:32 * k + 32, 3::4] = wm[:, :, 1, k].T  # j=3, kh=1
    wt[6, 0:32, 3::4] = wm[:, :, 1, 4].T                    # row+2 @ kw=4
    for s in range(1, 4):  # wB slots 1..3: row+3 @ kw=s-1
        wt[6, 32 * s:32 * s + 32, 3::4] = wm[:, :, 2, s - 1].T  # j=3, kh=2
    return wt


def _build_host_tiles(xc, width):
    """Host-side tile images for one latitude chunk, [3, 128, 8, width+4]
    bf16: stream 0 = x4 (window-1), 1 = x2a, 2 = x2b.

    SBUF position b*W+u of stream t holds ht[t, q, b, u]; the matmul
    reads block b at positions [2+b*W, 2+b*W+width).
      x4  slot rp: pos u <- x[row 4b+rp-2, u-2]      (kw via rhs offset)
      x2a slot k : pos u <- x[row 4b+2,   u+k-4]     (kw=k baked)
      x2b slot 0 : pos u <- x[row 4b+2,   u+0]      (kw=4 baked)
      x2b slot s : pos u <- x[row 4b+3,   u+s-5]    (kw=s-1 baked)
    """
    w4 = width + 4
    ht = np.zeros((3, 128, NBLK, w4), dtype=NPBF16)
    # chunk rows padded by 2 zero rows on top (per-chunk SAME padding)
    xcp = np.concatenate(
        [np.zeros((CI, 2, width), dtype=NPBF16), xc], axis=1)
    bidx = 4 * np.arange(NBLK)
    for rp in range(4):
        # rows 4b+rp-2 -> padded index 4b+rp
        ht[0, 32 * rp:32 * rp + 32, :, 2:2 + width] = \
            xcp[:, bidx + rp, :].transpose(0, 1, 2)
    r2 = xc[:, bidx + 2, :]  # [CI, NBLK, width]
    r3 = xc[:, bidx + 3, :]
    for k in range(4):
        ht[1, 32 * k:32 * k + 32, :, 4 - k:4 - k + width] = r2
    ht[2, 0:32, :, 0:width] = r2                   # kw=4
    for s in range(1, 4):
        ht[2, 32 * s:32 * s + 32, :, 5 - s:5 - s + width] = r3  # kw=s-1
    return ht


def _tile_groups(width):
    """Split the 8 output blocks of a chunk into PSUM tile groups of k
    blocks, keeping k*width <= 512 (one PSUM bank)."""
    if width >= 256:
        return [(b, 1) for b in range(NBLK)]
    k = min(NBLK, 512 // width)
    groups = []
    b = 0
    while b < NBLK:
        kk = min(k, NBLK - b)
        groups.append((b, kk))
        b += kk
    return groups


def _build_program(widths, has_bias):
    nc = bacc.Bacc("TRN2", target_bir_lowering=False, debug=False,
                   num_devices=NCORES)

    offs = [0]
    for wv in widths:
        offs.append(offs[-1] + NBLK * (wv + 4))
    tot = offs[-1]

    hx_d = [nc.dram_tensor(f"hx{t}", [128, tot], BF16, kind="ExternalInput")
            for t in range(3)]
    wt_d = nc.dram_tensor("wt", [NW, 128, 128], BF16, kind="ExternalInput")
    alpha_d = nc.dram_tensor("alpha_p", [128, 1], F32, kind="ExternalInput")
    if has_bias:
        bias_d = nc.dram_tensor("bias_p", [128, 1], F32, kind="ExternalInput")
    y_d = nc.dram_tensor("y", [CO, H, W], BF16, kind="ExternalOutput")

    with tile.TileContext(nc) as tc, ExitStack() as ctx:
        wpool = ctx.enter_context(tc.tile_pool(name="wts", bufs=1))
        spool = ctx.enter_context(tc.tile_pool(name="scalars", bufs=1))
        xpool = ctx.enter_context(tc.tile_pool(name="xt", bufs=4))
        psumpool = ctx.enter_context(
            tc.tile_pool(name="psum", bufs=8, space=bass.MemorySpace.PSUM))
        outpool = ctx.enter_context(tc.tile_pool(name="outsb", bufs=12))
        azpool = ctx.enter_context(tc.tile_pool(name="azp", bufs=4))

        wt = wpool.tile([128, NW, 128], BF16, tag="wt")
        nc.scalar.dma_start(wt[:, :, :],
                            wt_d.ap().rearrange("k q m -> q k m"))
        alpha_t = spool.tile([128, 1], F32, tag="alpha")
        nc.sync.dma_start(alpha_t[:], alpha_d.ap())
        if has_bias:
            bias_t = spool.tile([128, 1], F32, tag="bias")
            nc.sync.dma_start(bias_t[:], bias_d.ap())
        # dummy activation up front so the lazy ACT_TABLE_LOAD (~1.3us)
        # happens during the initial DMA wait, not at first postproc
        warm_t = spool.tile([128, 1], F32, tag="warm")
        nc.scalar.activation(warm_t[:, :], alpha_t[:, :],
                             mybir.ActivationFunctionType.Prelu,
                             bias=0.0, scale=1.0, alpha=alpha_t[:, :])

        prev_mm = [None]
        store_cnt = [0]
        prelu_cnt = [0]

        # small chunks first (cheap compute while the load pipeline
        # fills), large chunks last (tile-major drain hides postproc)
        chunk_order = [0, 7, 1, 6, 2, 5, 3, 4]
        for pi, p in enumerate(chunk_order):
            width = widths[p]
            w4 = width + 4
            xt = [xpool.tile([128, XLEN], BF16, tag=f"x{t}",
                             name=f"xt{t}")
                  for t in range(3)]
            engs = (nc.gpsimd, nc.sync, nc.scalar)
            for t in range(3):
                dst = xt[t][:, 0:NBLK * W].rearrange(
                    "q (b x) -> q b x", x=W)[:, :, 0:w4]
                src = hx_d[t].ap()[:, offs[p]:offs[p + 1]].rearrange(
                    "q (b x) -> q b x", x=w4)
                engs[(3 * pi + t) % 3].dma_start(dst, src)

            all_groups = _tile_groups(width)
            if pi >= NPART - 2:
                # tail chunks: tile-major so postproc drains immediately
                halves = [[g] for g in all_groups]
            elif len(all_groups) >= 6:
                halves = [all_groups[0:3], all_groups[3:6], all_groups[6:]]
            else:
                halves = [all_groups[:(len(all_groups) + 1) // 2],
                          all_groups[(len(all_groups) + 1) // 2:]]

            for groups in halves:
              if not groups:
                  continue
              psums = []
              for (b0, k) in groups:
                ps_t = psumpool.tile([128, k * width], F32, tag="ps")
                psums.append(ps_t)

              # weight-major: reuse each stationary weight across all
              # groups back-to-back (LDWEIGHTS ~106ns hides behind the
              # previous matmul's column stream)
              for wi in range(NW):
                if wi < KSIZE:
                    lhsT = wt[:, wi, :]
                else:
                    lhsT = wt[:, wi, :]
                for gi, (b0, k) in enumerate(groups):
                    if wi < KSIZE:
                        s = b0 * W + wi
                        src_t = xt[0]
                    else:
                        s = 2 + b0 * W
                        src_t = xt[wi - 4]  # wi=5 -> x2a, wi=6 -> x2b
                    rhs = src_t[:, s:s + k * W].rearrange(
                        "q (b x) -> q b x", x=W)[:, :, 0:width]
                    pview = psums[gi][:, :].rearrange(
                        "q (b x) -> q b x", x=width)
                    mm = nc.tensor.matmul(
                        pview,
                        lhsT,
                        rhs,
                        start=(wi == 0),
                        stop=(wi == NW - 1),
                    )
                    if prev_mm[0] is not None:
                        bass._add_dep_helper(
                            mm.ins, prev_mm[0].ins, sync=False,
                            reason="pe-stream-order")
                    prev_mm[0] = mm

              # postproc: PReLU each PSUM tile into SBUF, one store per
              # 4-row block (DMA AP balancing caps patterns at 3 dims)
              for gi, (b0, k) in enumerate(groups):
                n = k * width
                out_t = outpool.tile([128, n], BF16, tag="osb")
                prelu_cnt[0] += 1
                if has_bias or prelu_cnt[0] % 3 != 0:
                    # single ACT op: out = prelu(psum + bias, alpha)
                    nc.scalar.activation(
                        out_t[:, :], psums[gi][:, :],
                        mybir.ActivationFunctionType.Prelu,
                        bias=(bias_t[:, :] if has_bias else 0.0),
                        scale=1.0, alpha=alpha_t[:, :])
                else:
                    # DVE path (no bias): out = max(alpha*psum, psum)
                    az = azpool.tile([128, n], F32, tag="az")
                    nc.vector.tensor_copy(az[:, :], psums[gi][:, :])
                    nc.vector.scalar_tensor_tensor(
                        out_t[:, :], az[:, :], alpha_t[:, :], az[:, :],
                        mybir.AluOpType.mult, mybir.AluOpType.max)
                for bb in range(k):
                    hb = p * NBLK + b0 + bb
                    dst = y_d.ap()[:, 4 * hb:4 * hb + 4, 0:width]
                    store_cnt[0] += 1
                    if pi >= NPART - 2:
                        # drain tail stores on three queues
                        eng = (nc.sync, nc.scalar,
                               nc.gpsimd)[store_cnt[0] % 3]
                    else:
                        eng = (nc.sync, nc.gpsimd)[store_cnt[0] % 2]
                    eng.dma_start(dst,
                                  out_t[:, bb * width:(bb + 1) * width])

    nc.compile()
    return nc


def kernel(x, weight, bias, alpha, widths, _trace=False):
    global LAST_RESULT
    x = np.asarray(x, dtype=np.float32)
    weight = np.asarray(weight, dtype=np.float32)
    bias = np.asarray(bias, dtype=np.float32)
    alpha = np.asarray(alpha, dtype=np.float32)
    widths_np = np.asarray(widths, dtype=np.int32)
    wlist = [int(v) for v in widths_np]
    assert x.shape == (B, CI, H, W)
    for wv in wlist:
        assert 4 <= wv <= W - 6 and wv % 2 == 0, \
            f"width {wv} outside supported range"

    wt = _build_weights(weight)
    alpha_p = np.ascontiguousarray(
        np.repeat(alpha, 4)[:, None].astype(np.float32))
    has_bias = bool(np.any(bias != 0.0))

    nc = _build_program(wlist, has_bias)

    x_bf = x.astype(NPBF16)
    shared = {
        "wt": np.ascontiguousarray(wt.astype(NPBF16)),
        "alpha_p": alpha_p,
    }
    if has_bias:
        shared["bias_p"] = np.ascontiguousarray(
            np.repeat(bias, 4)[:, None].astype(np.float32))

    in_maps = []
    for b in range(B):
        streams = [[], [], []]
        for p in range(NPART):
            wv = wlist[p]
            xc = x_bf[b, :, p * Hp:(p + 1) * Hp, 0:wv]
            ht = _build_host_tiles(xc, wv)
            for t in range(3):
                streams[t].append(ht[t].reshape(128, -1))
        m = dict(shared)
        for t in range(3):
            m[f"hx{t}"] = np.ascontiguousarray(
                np.concatenate(streams[t], axis=1))
        in_maps.append(m)

    res = run_bass_kernel_spmd(nc, in_maps, list(range(NCORES)),
                               trace=_trace)
    LAST_RESULT = res
    y = np.stack([np.asarray(res.results[c]["y"]).astype(np.float32)
                  for c in range(NCORES)], axis=0)
    return y


if __name__ == "__main__":
    # smoke test with random data (no reference comparison)
    rng = np.random.default_rng(0)
    x = rng.standard_normal((B, CI, H, W), dtype=np.float32)
    weight = (rng.standard_normal((CO, CI, 5, 5)) * 0.05).astype(np.float32)
    bias = np.zeros(CO, np.float32)
    alpha = np.full(CO, 0.25, np.float32)
    lat = (np.arange(NPART) + 0.5) / NPART * np.pi - np.pi / 2.0
    widths = np.maximum(((np.cos(lat) * W).astype(np.int32) // 2) * 2, 16)
    y = kernel(x, weight, bias, alpha, widths.astype(np.int32))
    print("out", y.shape, y.dtype, float(np.abs(y).max()))


# revision 33
# speedup vs baseline: 1.0900x; 1.0900x over previous
"""Trainium2 Bass kernel for nn_EntropyConv (masked 5x5 PixelCNN-style conv,
per-latitude-partition padding + width masking + PReLU).

Strategy: data-parallel over batch (8 cores x 1 batch element). Per core,
a row-phase-split SBUF layout puts (row mod 4, ci) on the 128 K-partitions
so each PSUM tile computes 4 output rows x 32 channels. The PixelCNN mask
zeroes kh=3,4 entirely, so each output row only needs input rows r-2..r.
Window-1 (rows 4b-2..4b+1) takes 5 kw-shift matmuls; the window-2 taps
(rows 4b+2,4b+3 -> 11 weight blocks) are packed into just 2 matmuls using
duplicate tiles that bake the kw column shift into the storage offset,
for 7 matmuls per tile instead of 10 (the packing floor: 28 distinct
(row, kw) slot-pairs / 4 slots per matmul).

All tile images (x4 window-1, x2a/x2b window-2 with baked shifts, guard
zeros included) are assembled on the host in bf16, so the device issues
only 3 input DMAs per latitude chunk. Matmuls are bf16 (PSUM fp32);
output is stored bf16 and upcast on host (rel tolerance 2e-2).
"""

import sys
from contextlib import ExitStack

import numpy as np
import ml_dtypes

sys.path.insert(0, "/opt/trn_rl_repo")

import concourse.bass as bass  # noqa: E402
import concourse.tile as tile  # noqa: E402
from concourse import bacc, mybir  # noqa: E402
from concourse.bass_utils import run_bass_kernel_spmd  # noqa: E402

# Model constants (hardcoded per problem spec)
NGROUPS, CIN, COUT, KSIZE, NPART = 8, 4, 4, 5, 8
B, H, W = 8, 256, 512
CI = NGROUPS * CIN   # 32
CO = NGROUPS * COUT  # 32
Hp = H // NPART      # 32 rows per latitude chunk
NBLK = Hp // 4       # 8 four-row blocks per chunk
NCORES = 8
F32 = mybir.dt.float32
BF16 = mybir.dt.bfloat16
FP8 = mybir.dt.float8e4
NPBF16 = ml_dtypes.bfloat16
NPFP8 = ml_dtypes.float8_e4m3

# all streams bf16: fp8e4m3 (3 mantissa bits, ~3.6% RMS/elem) blows the
# 2e-2 max-abs budget even on the small window-2 tap subset
STREAM_DT = (BF16, BF16, BF16)
STREAM_NP = (NPBF16, NPBF16, NPBF16)

NW = 7  # matmuls per 4-row tile: 5 window-1 kw shifts + 2 packed window-2

LAST_RESULT = None  # BassKernelResults from the most recent run (for test.py)


def _group_mask():
    """PixelCNN group mask for 5x5 kernel, mask-B (hidden) variant."""
    m = np.zeros((CO, CI, KSIZE, KSIZE), np.float32)
    c = KSIZE // 2
    m[:, :, :c, :] = 1.0
    m[:, :, c, :c] = 1.0
    gin = np.arange(CI) // CIN
    gout = np.arange(CO) // COUT
    center = gin[None, :] <= gout[:, None]
    m[:, :, c, c] = center.astype(np.float32)
    return m


def _build_weights(weight):
    """lhsT stack [7, 128, 128]: 5 window-1 block-Toeplitz matrices (one
    per kw) + 2 packed window-2 matrices.

    w[kw, 32*rp+ci, 4*co+j]: input row (4hb+rp-2) -> output row (4hb+j),
    kh = rp-j (mask zeroes kh >= 3).
    wA slot k (k=0..3) = row 4hb+2 @ kw=k:  j=2 <- wm[2,k], j=3 <- wm[1,k]
    wB slot 0          = row 4hb+2 @ kw=4:  j=3 <- wm[1,4]
    wB slot s (s=1..3) = row 4hb+3 @ kw=s-1: j=3 <- wm[2,s-1]
    """
    wm = (weight * _group_mask()).astype(np.float32)  # [co, ci, kh, kw]
    wt = np.zeros((NW, 128, 128), np.float32)
    for kw in range(KSIZE):
        for rp in range(4):
            for j in range(4):
                kh = rp - j
                if 0 <= kh < KSIZE:
                    wt[kw, 32 * rp:32 * rp + 32, j::4] = wm[:, :, kh, kw].T
    for k in range(4):  # wA slots: row+2 @ kw=k
        wt[5, 32 * k:32 * k + 32, 2::4] = wm[:, :, 2, k].T  # j=2, kh=2
        wt[5, 32 * k:32 * k + 32, 3::4] = wm[:, :, 1, k].T  # j=3, kh=1
    wt[6, 0:32, 3::4] = wm[:, :, 1, 4].T                    # row+2 @ kw=4
    for s in range(1, 4):  # wB slots 1..3: row+3 @ kw=s-1
        wt[6, 32 * s:32 * s + 32, 3::4] = wm[:, :, 2, s - 1].T  # j=3, kh=2
    return wt


def _build_host_tiles(xc, width):
    """Host-side tile images for one latitude chunk, [3, 128, 8, width+4]
    bf16: stream 0 = x4 (window-1), 1 = x2a, 2 = x2b.

    SBUF position b*W+u of stream t holds ht[t, q, b, u]; the matmul
    reads block b at positions [2+b*W, 2+b*W+width).
      x4  slot rp: pos u <- x[row 4b+rp-2, u-2]      (kw via rhs offset)
      x2a slot k : pos u <- x[row 4b+2,   u+k-4]     (kw=k baked)
      x2b slot 0 : pos u <- x[row 4b+2,   u+0]      (kw=4 baked)
      x2b slot s : pos u <- x[row 4b+3,   u+s-5]    (kw=s-1 baked)
    """
    w4 = width + 4
    ht = np.zeros((3, 128, NBLK, w4), dtype=NPBF16)
    # chunk rows padded by 2 zero rows on top (per-chunk SAME padding)
    xcp = np.concatenate(
        [np.zeros((CI, 2, width), dtype=NPBF16), xc], axis=1)
    bidx = 4 * np.arange(NBLK)
    for rp in range(4):
        # rows 4b+rp-2 -> padded index 4b+rp
        ht[0, 32 * rp:32 * rp + 32, :, 2:2 + width] = \
            xcp[:, bidx + rp, :].transpose(0, 1, 2)
    r2 = xc[:, bidx + 2, :]  # [CI, NBLK, width]
    r3 = xc[:, bidx + 3, :]
    for k in range(4):
        ht[1, 32 * k:32 * k + 32, :, 4 - k:4 - k + width] = r2
    ht[2, 0:32, :, 0:width] = r2                   # kw=4
    for s in range(1, 4):
        ht[2, 32 * s:32 * s + 32, :, 5 - s:5 - s + width] = r3  # kw=s-1
    return ht


def _tile_groups(width):
    """Split the 8 output blocks of a chunk into PSUM tile groups of k
    blocks, keeping k*width <= 512 (one PSUM bank)."""
    if width >= 256:
        return [(b, 1) for b in range(NBLK)]
    k = min(NBLK, 512 // width)
    groups = []
    b = 0
    while b < NBLK:
        kk = min(k, NBLK - b)
        groups.append((b, kk))
        b += kk
    return groups


def _build_program(widths, has_bias):
    nc = bacc.Bacc("TRN2", target_bir_lowering=False, debug=False,
                   num_devices=NCORES)

    offs = [0]
    for wv in widths:
        offs.append(offs[-1] + NBLK * (wv + 4))
    tot = offs[-1]

    hx_d = [nc.dram_tensor(f"hx{t}", [128, tot], STREAM_DT[t],
                           kind="ExternalInput")
            for t in range(3)]
    wt_d = nc.dram_tensor("wt", [NW, 128, 128], BF16,
                          kind="ExternalInput")
    alpha_d = nc.dram_tensor("alpha_p", [128, 1], F32, kind="ExternalInput")
    if has_bias:
        bias_d = nc.dram_tensor("bias_p", [128, 1], F32, kind="ExternalInput")
    y_d = nc.dram_tensor("y", [CO, H, W], BF16, kind="ExternalOutput")

    with tile.TileContext(nc) as tc, ExitStack() as ctx:
        wpool = ctx.enter_context(tc.tile_pool(name="wts", bufs=1))
        spool = ctx.enter_context(tc.tile_pool(name="scalars", bufs=1))
        xpool = ctx.enter_context(tc.tile_pool(name="xt", bufs=4))
        psumpool = ctx.enter_context(
            tc.tile_pool(name="psum", bufs=8, space=bass.MemorySpace.PSUM))
        outpool = ctx.enter_context(tc.tile_pool(name="outsb", bufs=12))
        azpool = ctx.enter_context(tc.tile_pool(name="azp", bufs=4))

        wts = wpool.tile([128, NW, 128], BF16, tag="wts")
        nc.scalar.dma_start(wts[:, :, :],
                            wt_d.ap().rearrange("k q m -> q k m"))
        alpha_t = spool.tile([128, 1], F32, tag="alpha")
        nc.sync.dma_start(alpha_t[:], alpha_d.ap())
        if has_bias:
            bias_t = spool.tile([128, 1], F32, tag="bias")
            nc.sync.dma_start(bias_t[:], bias_d.ap())
        # dummy activation up front so the lazy ACT_TABLE_LOAD (~1.3us)
        # happens during the initial DMA wait, not at first postproc
        warm_t = spool.tile([128, 1], F32, tag="warm")
        nc.scalar.activation(warm_t[:, :], alpha_t[:, :],
                             mybir.ActivationFunctionType.Prelu,
                             bias=0.0, scale=1.0, alpha=alpha_t[:, :])

        prev_mm = [None]
        store_cnt = [0]
        prelu_cnt = [0]

        # descending width: ~3.5us initial load wait, then each chunk's
        # loads (~0.7x its compute) hide fully; tiny 98s drain at the end
        chunk_order = [3, 4, 2, 5, 1, 6, 0, 7]
        for pi, p in enumerate(chunk_order):
            width = widths[p]
            w4 = width + 4  # tight block pitch (guards baked by host)
            xt = []
            for t in range(3):
                # +8 slack: shifted k-group slices may extend past the
                # last block (the [:, :, 0:width] view never reads there)
                xt.append(xpool.tile([128, NBLK * w4 + 8], STREAM_DT[t],
                                     tag=f"x{t}", name=f"xt{t}"))
            engs = (nc.gpsimd, nc.sync, nc.scalar)
            for t in range(3):
                # fully contiguous per partition: 128 descriptors
                engs[t].dma_start(xt[t][:, 0:NBLK * w4],
                                  hx_d[t].ap()[:, offs[p]:offs[p + 1]])

            all_groups = _tile_groups(width)
            if pi >= NPART - 2:
                # tail chunks: tile-major so postproc drains immediately
                halves = [[g] for g in all_groups]
            elif len(all_groups) >= 6:
                halves = [all_groups[0:3], all_groups[3:6], all_groups[6:]]
            else:
                halves = [all_groups[:(len(all_groups) + 1) // 2],
                          all_groups[(len(all_groups) + 1) // 2:]]

            for groups in halves:
              if not groups:
                  continue
              psums = []
              for (b0, k) in groups:
                ps_t = psumpool.tile([128, k * width], F32, tag="ps")
                psums.append(ps_t)

              # weight-major: reuse each stationary weight across all
              # groups back-to-back (LDWEIGHTS ~106ns hides behind the
              # previous matmul's column stream)
              for wi in range(NW):
                lhsT = wts[:, wi, :]
                for gi, (b0, k) in enumerate(groups):
                    if wi < KSIZE:
                        s = b0 * w4 + wi
                        src_t = xt[0]
                    else:
                        s = 2 + b0 * w4
                        src_t = xt[wi - 4]  # wi=5 -> x2a, wi=6 -> x2b
                    rhs = src_t[:, s:s + k * w4].rearrange(
                        "q (b x) -> q b x", x=w4)[:, :, 0:width]
                    pview = psums[gi][:, :].rearrange(
                        "q (b x) -> q b x", x=width)
                    mm = nc.tensor.matmul(
                        pview,
                        lhsT,
                        rhs,
                        start=(wi == 0),
                        stop=(wi == NW - 1),
                    )
                    if prev_mm[0] is not None:
                        bass._add_dep_helper(
                            mm.ins, prev_mm[0].ins, sync=False,
                            reason="pe-stream-order")
                    prev_mm[0] = mm

              # postproc: PReLU each PSUM tile into SBUF, one store per
              # 4-row block (DMA AP balancing caps patterns at 3 dims)
              for gi, (b0, k) in enumerate(groups):
                n = k * width
                out_t = outpool.tile([128, n], BF16, tag="osb")
                prelu_cnt[0] += 1
                if has_bias or prelu_cnt[0] % 3 != 0:
                    # single ACT op: out = prelu(psum + bias, alpha)
                    nc.scalar.activation(
                        out_t[:, :], psums[gi][:, :],
                        mybir.ActivationFunctionType.Prelu,
                        bias=(bias_t[:, :] if has_bias else 0.0),
                        scale=1.0, alpha=alpha_t[:, :])
                else:
                    # DVE path (no bias): out = max(alpha*psum, psum)
                    az = azpool.tile([128, n], F32, tag="az")
                    nc.vector.tensor_copy(az[:, :], psums[gi][:, :])
                    nc.vector.scalar_tensor_tensor(
                        out_t[:, :], az[:, :], alpha_t[:, :], az[:, :],
                        mybir.AluOpType.mult, mybir.AluOpType.max)
                for bb in range(k):
                    hb = p * NBLK + b0 + bb
                    dst = y_d.ap()[:, 4 * hb:4 * hb + 4, 0:width]
                    store_cnt[0] += 1
                    if pi >= NPART - 2:
                        # drain tail stores on three queues
                        eng = (nc.sync, nc.scalar,
                               nc.gpsimd)[store_cnt[0] % 3]
                    else:
                        eng = (nc.sync, nc.gpsimd)[store_cnt[0] % 2]
                    eng.dma_start(dst,
                                  out_t[:, bb * width:(bb + 1) * width])

    nc.compile()
    return nc


def kernel(x, weight, bias, alpha, widths, _trace=False):
    global LAST_RESULT
    x = np.asarray(x, dtype=np.float32)
    weight = np.asarray(weight, dtype=np.float32)
    bias = np.asarray(bias, dtype=np.float32)
    alpha = np.asarray(alpha, dtype=np.float32)
    widths_np = np.asarray(widths, dtype=np.int32)
    wlist = [int(v) for v in widths_np]
    assert x.shape == (B, CI, H, W)
    for wv in wlist:
        assert 4 <= wv <= W - 6 and wv % 2 == 0, \
            f"width {wv} outside supported range"

    wt = _build_weights(weight)
    alpha_p = np.ascontiguousarray(
        np.repeat(alpha, 4)[:, None].astype(np.float32))
    has_bias = bool(np.any(bias != 0.0))

    nc = _build_program(wlist, has_bias)

    x_bf = x.astype(NPBF16)
    shared = {
        "wt": np.ascontiguousarray(wt.astype(NPBF16)),
        "alpha_p": alpha_p,
    }
    if has_bias:
        shared["bias_p"] = np.ascontiguousarray(
            np.repeat(bias, 4)[:, None].astype(np.float32))

    in_maps = []
    for b in range(B):
        streams = [[], [], []]
        for p in range(NPART):
            wv = wlist[p]
            xc = x_bf[b, :, p * Hp:(p + 1) * Hp, 0:wv]
            ht = _build_host_tiles(xc, wv)
            for t in range(3):
                streams[t].append(ht[t].reshape(128, -1))
        m = dict(shared)
        for t in range(3):
            m[f"hx{t}"] = np.ascontiguousarray(
                np.concatenate(streams[t], axis=1).astype(STREAM_NP[t]))
        in_maps.append(m)

    res = run_bass_kernel_spmd(nc, in_maps, list(range(NCORES)),
                               trace=_trace)
    LAST_RESULT = res
    y = np.stack([np.asarray(res.results[c]["y"]).astype(np.float32)
                  for c in range(NCORES)], axis=0)
    return y


if __name__ == "__main__":
    # smoke test with random data (no reference comparison)
    rng = np.random.default_rng(0)
    x = rng.standard_normal((B, CI, H, W), dtype=np.float32)
    weight = (rng.standard_normal((CO, CI, 5, 5)) * 0.05).astype(np.float32)
    bias = np.zeros(CO, np.float32)
    alpha = np.full(CO, 0.25, np.float32)
    lat = (np.arange(NPART) + 0.5) / NPART * np.pi - np.pi / 2.0
    widths = np.maximum(((np.cos(lat) * W).astype(np.int32) // 2) * 2, 16)
    y = kernel(x, weight, bias, alpha, widths.astype(np.int32))
    print("out", y.shape, y.dtype, float(np.abs(y).max()))
